# revision 18
# baseline (speedup 1.0000x reference)
"""Multi-head attention (B=4, S=2048, D=1024, H=16) on 8 trn2 cores.

Sharding: core c -> batch b = c//2, head-half = c%2 (8 heads = 512 dims).
Each core computes attention for its (batch, 8 heads) and a partial output
projection over its 512 d-features; the host sums the two partials per batch
and adds the (bo + bv @ Wo.T) constant row vector.

Single fused pipeline per core (all matmul inputs bf16, host-cast):
  - x resident in SBUF as 8 [128, 2048] bf16 tiles.
  - Scores: per (k-tile, head-pair) one [128, 1024] PSUM region filled by a
    row-tiled matmul pair (head A on PE rows 0-63, head B on rows 64-127,
    concurrent on HW) -> one exp activation per region (ACT is the pacing
    engine at ~285us busy).
  - AV: [65, 512] PSUM accumulation per head over 16 k-tiles (V columns plus
    a bf16 mask column that yields the softmax denominator); normalize via
    DVE copy + reciprocal + GpSimd partition broadcast + DVE multiply.
  - K/Q/V/out projections are emitted at lower scheduler priority than the
    attention stream; the Tile list scheduler uses them to fill PE stalls,
    keeping the PE HAM-warm. The first attention block is chunk-interleaved
    with its own producers so the exp stream starts ~8us in.
"""

import numpy as np
import ml_dtypes
from contextlib import ExitStack

import concourse.bacc as bacc
import concourse.tile as tile
import concourse.mybir as mybir
from concourse.bass_utils import run_bass_kernel_spmd

F32 = mybir.dt.float32
BF16 = mybir.dt.bfloat16
EXP = mybir.ActivationFunctionType.Exp

S = 2048          # sequence length
D = 1024          # model dim
HD = 64           # head dim
HP = 4            # head pairs per core (128 dims each)
DLOC = 512        # d-features per core
ET = D // 128     # 8 contraction tiles over D
NT = S // 128     # 16 k-tiles
QC = S // 512     # 4 query chunks of 512
VW = HD + 1       # V block width per head incl. mask column


def build_nc():
    nc = bacc.Bacc(None)
    xT = nc.dram_tensor("xT", [D, S], BF16, kind="ExternalInput")
    wqT = nc.dram_tensor("wqT", [D, DLOC], BF16, kind="ExternalInput")
    wkT = nc.dram_tensor("wkT", [D, DLOC], BF16, kind="ExternalInput")
    wvT = nc.dram_tensor("wvT", [D, DLOC], BF16, kind="ExternalInput")
    woT = nc.dram_tensor("woT", [DLOC, D], BF16, kind="ExternalInput")
    bq = nc.dram_tensor("bq", [DLOC, 1], F32, kind="ExternalInput")
    bk = nc.dram_tensor("bk", [DLOC, 1], F32, kind="ExternalInput")
    msk = nc.dram_tensor("msk", [S, 1], F32, kind="ExternalInput")
    out = nc.dram_tensor("out", [S, D], F32, kind="ExternalOutput")

    with tile.TileContext(nc) as tc, ExitStack() as ctx:
        res = ctx.enter_context(tc.tile_pool(name="res", bufs=1))

        kt = [res.tile([128, S], BF16, tag=f"kt{i}", name=f"kt{i}") for i in range(HP)]
        qt = [res.tile([128, S], BF16, tag=f"qt{i}", name=f"qt{i}") for i in range(HP)]
        valsT = [res.tile([128, S], BF16, tag=f"valsT{i}", name=f"valsT{i}")
                 for i in range(HP)]
        # V per half (heads 4h..4h+3), 65 cols per head (64 V dims + mask)
        vm = [[res.tile([128, 4 * VW], BF16, tag=f"vm{h}_{t}", name=f"vm{h}_{t}")
               for t in range(NT)] for h in range(2)]
        xt_all = res.tile([128, ET * S], BF16, tag="xt_all")
        xt = [xt_all[:, e * S:(e + 1) * S] for e in range(ET)]
        wk_all = res.tile([128, ET * DLOC], BF16, tag="wk_all")
        wk_sb = [wk_all[:, e * DLOC:(e + 1) * DLOC] for e in range(ET)]
        wq_all = res.tile([128, ET * DLOC], BF16, tag="wq_all")
        wq_sb = [wq_all[:, e * DLOC:(e + 1) * DLOC] for e in range(ET)]
        wv_all = res.tile([128, ET * DLOC], BF16, tag="wv_all")
        wv_sb = [wv_all[:, e * DLOC:(e + 1) * DLOC] for e in range(ET)]
        wo_all = res.tile([128, HP * D], BF16, tag="wo_all")
        wo_sb = [wo_all[:, i * D:(i + 1) * D] for i in range(HP)]
        m_sb = res.tile([128, NT], F32, tag="m_sb")
        m_bf = res.tile([128, NT], BF16, tag="m_bf")
        bq_sb = res.tile([128, HP], F32, tag="bq_sb")
        bk_sb = res.tile([128, HP], F32, tag="bk_sb")
        scratch = res.tile([1, 4], F32, tag="scratch")

        psS = ctx.enter_context(tc.tile_pool(name="psS", bufs=2, space="PSUM"))
        psOp = ctx.enter_context(tc.tile_pool(name="psO", bufs=2, space="PSUM"))
        psP = ctx.enter_context(tc.tile_pool(name="psP", bufs=2, space="PSUM"))
        ptp = ctx.enter_context(tc.tile_pool(name="pt", bufs=8))
        otp = ctx.enter_context(tc.tile_pool(name="ot", bufs=2))
        dn1p = ctx.enter_context(tc.tile_pool(name="dn1", bufs=2))
        dnbp = ctx.enter_context(tc.tile_pool(name="dnb", bufs=2))

        # ---- input DMAs (packed; issue order ~ need order) ----
        xt3 = xt_all.rearrange("p (a s) -> p a s", a=ET)
        nc.sync.dma_start(out=wk_all.rearrange("p (a d) -> p a d", a=ET),
                          in_=wkT.rearrange("(a p) d -> p a d", p=128))
        nc.sync.dma_start(out=xt3[:, :, 0:512],
                          in_=xT[:, 0:512].rearrange("(a p) s -> p a s", p=128))
        nc.sync.dma_start(out=wq_all.rearrange("p (a d) -> p a d", a=ET),
                          in_=wqT.rearrange("(a p) d -> p a d", p=128))
        nc.sync.dma_start(out=m_sb, in_=msk.rearrange("(a p) o -> p (a o)", p=128))
        nc.sync.dma_start(out=bk_sb, in_=bk.rearrange("(a p) o -> p (a o)", p=128))
        nc.sync.dma_start(out=bq_sb, in_=bq.rearrange("(a p) o -> p (a o)", p=128))
        for cs in range(1, QC):
            csl = slice(cs * 512, (cs + 1) * 512)
            nc.sync.dma_start(out=xt3[:, :, csl],
                              in_=xT[:, csl].rearrange("(a p) s -> p a s", p=128))
        nc.sync.dma_start(out=wv_all.rearrange("p (a d) -> p a d", a=ET),
                          in_=wvT.rearrange("(a p) d -> p a d", p=128))
        nc.sync.dma_start(out=wo_all.rearrange("p (a d) -> p a d", a=HP),
                          in_=woT.rearrange("(a p) d -> p a d", p=128))

        # preload the exp activation table while the PE does projections
        nc.scalar.activation(scratch, bk_sb[0:1, :], EXP, scale=1.0)
        # bf16 mask staging so the vm mask-column copies are cast-free
        nc.vector.tensor_copy(m_bf, m_sb)

        def kqproj_single(hp, w_sb, b_sb, dst, cs, kind):
            """K or Q projection for head pair hp, one 512-query chunk."""
            hcols = slice(hp * 128, (hp + 1) * 128)
            csl = slice(cs * 512, (cs + 1) * 512)
            ps = psP.tile([128, 512], F32, tag="psP", name=f"pj{kind}{hp}_{cs}")
            for e in range(ET):
                nc.tensor.matmul(ps, w_sb[e][:, hcols], xt[e][:, csl],
                                 start=(e == 0), stop=(e == ET - 1))
            nc.vector.tensor_scalar_add(dst[hp][:, csl], ps, b_sb[:, hp:hp + 1])

        def vproj_t(t):
            """V projection (all 8 heads) for k-tile t, mask folded in."""
            ksl = slice(t * 128, (t + 1) * 128)
            ps = psP.tile([128, 512], F32, tag="psP", name=f"pv{t}")
            for e in range(ET):
                nc.tensor.matmul(ps, xt[e][:, ksl], wv_sb[e],
                                 start=(e == 0), stop=(e == ET - 1))
            mc = m_sb[:, t:t + 1]
            for h in range(8):
                half, lh = h // 4, h % 4
                nc.vector.tensor_scalar_mul(
                    vm[half][t][:, lh * VW:lh * VW + HD],
                    ps[:, h * HD:(h + 1) * HD], mc)
                nc.gpsimd.tensor_copy(
                    out=vm[half][t][:, lh * VW + HD:lh * VW + VW],
                    in_=m_bf[:, t:t + 1])

        def warmers(n, tag):
            """Independent scratch matmuls that keep the PE HAM window busy
            while real work is blocked (DMA waits / RAW-ordered producers)."""
            ps = psP.tile([128, 512], F32, tag="psP", name=f"warm{tag}")
            for i in range(n):
                nc.tensor.matmul(ps[:, 0:128], wk_all[:, 0:128],
                                 wk_all[:, 128:256], start=True, stop=True)

        def scores_exp(qc, hp, t):
            """Row-tiled score pair for one k-tile + its exp; returns pt tile."""
            csl = slice(qc * 512, (qc + 1) * 512)
            ksl = slice(t * 128, (t + 1) * 128)
            ps = psS.tile([128, 1024], F32, tag="psS", name=f"psS{qc}_{hp}_{t}")
            nc.tensor.matmul(ps[:, 0:512], kt[hp][0:64, ksl],
                             qt[hp][0:64, csl], start=True, stop=True)
            nc.tensor.matmul(ps[:, 512:1024], kt[hp][64:128, ksl],
                             qt[hp][64:128, csl], start=True, stop=True)
            p = ptp.tile([128, 1024], BF16, tag="pt", name=f"pt{qc}_{hp}_{t}")
            nc.scalar.activation(p, ps, EXP, scale=0.125)
            return p

        def av(qc, hp, t, p, psO):
            half = hp // 2
            for h2 in range(2):
                lh = 2 * hp + h2 - half * 4
                nc.tensor.matmul(psO[h2], vm[half][t][:, lh * VW:(lh + 1) * VW],
                                 p[:, h2 * 512:(h2 + 1) * 512],
                                 start=(t == 0), stop=(t == NT - 1))

        def normalize(qc, hp, psO):
            csl = slice(qc * 512, (qc + 1) * 512)
            for h2 in range(2):
                d1 = dn1p.tile([1, 512], F32, tag="dn1", name=f"d1{qc}_{hp}_{h2}")
                nc.vector.tensor_copy(d1, psO[h2][HD:VW, :])
                dr = dn1p.tile([1, 512], F32, tag="dn1r", name=f"dr{qc}_{hp}_{h2}")
                nc.vector.reciprocal_approx_fast(out=dr, in_=d1)
                db = dnbp.tile([HD, 512], F32, tag="dnb", name=f"db{qc}_{hp}_{h2}")
                nc.gpsimd.partition_broadcast(db, dr)
                nc.vector.tensor_mul(valsT[hp][h2 * HD:(h2 + 1) * HD, csl],
                                     psO[h2][0:HD, :], db)

        def outproj_j(qc, j, last_hp_deferred=False):
            """One query-row-tile of the output projection (2 psC groups).
            Returns a finisher callback when last_hp_deferred."""
            st = qc * 4 + j
            ssl = slice(st * 128, (st + 1) * 128)
            o = otp.tile([128, D], F32, tag="ot", name=f"ot{st}")
            psC = [psP.tile([128, 512], F32, tag="psP", name=f"pc{st}_{ec}")
                   for ec in range(2)]
            n_pre = HP - 1 if last_hp_deferred else HP
            for hp in range(n_pre):
                for ec in range(2):
                    esl = slice(ec * 512, (ec + 1) * 512)
                    nc.tensor.matmul(psC[ec], valsT[hp][:, ssl], wo_sb[hp][:, esl],
                                     start=(hp == 0), stop=(hp == HP - 1))

            def fin():
                for hp in range(n_pre, HP):
                    for ec in range(2):
                        esl = slice(ec * 512, (ec + 1) * 512)
                        nc.tensor.matmul(psC[ec], valsT[hp][:, ssl],
                                         wo_sb[hp][:, esl],
                                         start=(hp == 0), stop=(hp == HP - 1))
                for ec in range(2):
                    esl = slice(ec * 512, (ec + 1) * 512)
                    nc.vector.tensor_copy(o[:, esl], psC[ec])
                nc.sync.dma_start(out=out[ssl, :], in_=o)

            if last_hp_deferred:
                return fin
            fin()
            return None

        # ---- block schedule: wave 0 = head pairs 0,1; wave 1 = 2,3 ----
        blocks = ([(qc, hp) for qc in range(QC) for hp in (0, 1)]
                  + [(qc, hp) for qc in range(QC) for hp in (2, 3)])

        # producers[i] = {t: [callbacks]} emitted before that t of block i.
        producers = [dict() for _ in blocks]

        def addp(bi, t, fn):
            producers[bi].setdefault(t, []).append(fn)

        def kq(kind, hp, cs):
            w_sb, b_sb, dst = ((wk_sb, bk_sb, kt) if kind == "k"
                               else (wq_sb, bq_sb, qt))
            return lambda: kqproj_single(hp, w_sb, b_sb, dst, cs, kind)

        # block 0 (qc0, hp0): its own K chunks + V tiles inline (RAW order);
        # K cs c needed before t=4c, vm[t] needed before av at loop index t+2.
        for cs in range(1, QC):
            addp(0, 4 * cs, kq("k", 0, cs))
        for t in range(NT):
            addp(0, min(t + 2, NT - 1), (lambda tt: lambda: vproj_t(tt))(t))
        # block 1 (qc0, hp1): K/Q spread mid-block-0
        addp(0, 5, kq("k", 1, 0))
        addp(0, 7, kq("k", 1, 1))
        addp(0, 9, kq("k", 1, 2))
        addp(0, 11, kq("k", 1, 3))
        addp(0, 13, kq("q", 1, 0))
        # Q chunks for wave-0 blocks 2..7, one block ahead, mid-block
        for bi, (qc, hp) in enumerate(blocks[2:8], start=2):
            addp(bi - 1, 6, kq("q", hp, qc))
        # wave-1 K projections spread across wave-0 blocks 3..6
        for i, hp in enumerate((2, 2, 3, 3)):
            addp(3 + i, 3, kq("k", hp, 2 * (i % 2)))
            addp(3 + i, 9, kq("k", hp, 2 * (i % 2) + 1))
        # wave-1 Q chunks one block ahead
        for bi, (qc, hp) in enumerate(blocks[8:], start=8):
            addp(bi - 1, 6, kq("q", hp, qc))
        # wave-1 output projection: outproj(qc-1) spread inside blocks (qc,2/3)
        for qc in range(QC):
            bi = 8 + 2 * qc        # block (qc, 2)
            if qc > 0:
                for j in range(4):
                    addp(bi, 3 + 4 * j, (lambda q, jj: lambda: outproj_j(q, jj))(qc - 1, j))
        # HAM warmers at wave-1 qc-section boundaries (unfillable stalls)
        for qc in range(QC):
            addp(8 + 2 * qc, 2, (lambda q: lambda: warmers(6, f"w1_{q}"))(qc))
        # outproj(3) runs as the tail after the last block's normalize

        # ---- prologue ----
        kqproj_single(0, wk_sb, bk_sb, kt, 0, "k")
        kqproj_single(0, wq_sb, bq_sb, qt, 0, "q")
        warmers(24, "pro")

        # ---- streaming emission with AV lagging scores by 2 regions ----
        avq = []
        psO_of = {}

        def flush_av():
            bi_t = avq.pop(0)
            (bqc, bhp), t = bi_t
            if t == 0:
                psO_of[(bqc, bhp)] = [
                    psOp.tile([VW, 512], F32, tag="psO", name=f"psO{bqc}_{bhp}_{h2}")
                    for h2 in range(2)]
            av(bqc, bhp, t, pt_of[bi_t], psO_of[(bqc, bhp)])
            if t == NT - 1:
                normalize(bqc, bhp, psO_of.pop((bqc, bhp)))

        pt_of = {}
        for bi, (qc, hp) in enumerate(blocks):
            pr = producers[bi]
            for t in range(NT):
                for fn in pr.get(t, ()):
                    fn()
                pt_of[((qc, hp), t)] = scores_exp(qc, hp, t)
                if len(avq) >= 2:
                    flush_av()
                avq.append(((qc, hp), t))
        while avq:
            flush_av()
        for j in range(4):
            outproj_j(3, j)

    nc.finalize()
    return nc


_NC_CACHE = None


def _get_nc():
    global _NC_CACHE
    if _NC_CACHE is None:
        _NC_CACHE = build_nc()
    return _NC_CACHE


def _bf16(a):
    return np.ascontiguousarray(np.asarray(a).astype(ml_dtypes.bfloat16))


def make_in_maps(x, mask, Wq, bq, Wk, bk, Wv, Wo):
    in_maps = []
    for c in range(8):
        b = c // 2
        dsl = slice((c % 2) * DLOC, (c % 2) * DLOC + DLOC)
        in_maps.append({
            "xT": _bf16(np.asarray(x[b]).T),
            "wqT": _bf16(np.asarray(Wq)[dsl, :].T),
            "wkT": _bf16(np.asarray(Wk)[dsl, :].T),
            "wvT": _bf16(np.asarray(Wv)[dsl, :].T),
            "woT": _bf16(np.asarray(Wo)[:, dsl].T),
            "bq": np.ascontiguousarray(np.asarray(bq, dtype=np.float32)[dsl])[:, None],
            "bk": np.ascontiguousarray(np.asarray(bk, dtype=np.float32)[dsl])[:, None],
            "msk": np.asarray(mask[b], dtype=np.float32)[:, None],
        })
    return in_maps


def assemble(results, Wo, bo, bv):
    out = np.empty((4, S, D), dtype=np.float32)
    for b in range(4):
        out[b] = results[2 * b]["out"] + results[2 * b + 1]["out"]
    out += (np.asarray(bo) + np.asarray(bv) @ np.asarray(Wo).T).astype(np.float32)
    return out


def run(x, mask, Wq, bq, Wk, bk, Wv, bv, Wo, bo, trace=False):
    nc = _get_nc()
    in_maps = make_in_maps(x, mask, Wq, bq, Wk, bk, Wv, Wo)
    res = run_bass_kernel_spmd(nc, in_maps, list(range(8)), trace=trace)
    return assemble(res.results, Wo, bo, bv), res


def kernel(x, mask, Wq, bq, Wk, bk, Wv, bv, Wo, bo):
    out, _ = run(x, mask, Wq, bq, Wk, bk, Wv, bv, Wo, bo)
    return out
